# revision 16
# baseline (speedup 1.0000x reference)
"""Trainium2 Bass kernel for PolyIoULoss — 6-edge x-dy slab-clip formulation.

Green's theorem in the single form A = oint x dy, pred frame: pred edges
y=+-b contribute nothing (dy=0), so only 6 of 8 edges are clipped.
Uniform numerator structure: for each slab family the two opposite edges'
interval centers are M_even = (Q - U)*I, M_odd = (Q + U)*I with shared
reciprocal I and shared alpha = |h*I|.  Rotation convention matches the
reference (corners right-multiplied by R, i.e. theta -> -theta): both
sine slots hold -sin(theta).

Per core, 65536 pairs as [128 x 512], pipelined in two F-halves.
Sharding: data-parallel over 8 cores; host sums the [P,2] partials.
"""

import numpy as np

import concourse.bacc as bacc
import concourse.tile as tile
from concourse import mybir
from concourse.mybir import AluOpType as Op, ActivationFunctionType as Fn
from concourse.bass_utils import run_bass_kernel_spmd

N_TOTAL = 524288
NCORES = 8
NPER = N_TOTAL // NCORES
P = 128
F = NPER // P                     # 512
FH = F // 2                       # 256

HPI = float(np.pi / 2)
EPS = 1e-6
LNEPS = float(np.log(EPS))
F32 = mybir.dt.float32
BF16 = mybir.dt.bfloat16


def _build():
    nc = bacc.Bacc(None, target_bir_lowering=False)
    pred_h = nc.dram_tensor("pred", [NPER, 5], F32, kind="ExternalInput")
    tgt_h = nc.dram_tensor("target", [NPER, 5], F32, kind="ExternalInput")
    out_h = nc.dram_tensor("out", [P, 4], F32, kind="ExternalOutput")

    V = nc.vector
    G = nc.gpsimd
    A = nc.scalar

    with tile.TileContext(nc) as tc:
        with tc.tile_pool(name="main", bufs=1) as pool:
            PRED = pool.tile([P, F, 5], F32, tag="PRED")
            TGT = pool.tile([P, F, 5], F32, tag="TGT")
            predr = pred_h[:, :].rearrange("(p f) c -> p f c", p=P)
            tgtr = tgt_h[:, :].rearrange("(p f) c -> p f c", p=P)
            FQ = F // 4
            for q in range(0, 4):
                nc.sync.dma_start(out=PRED[:, q*FQ:(q+1)*FQ, :],
                                  in_=predr[:, q*FQ:(q+1)*FQ, :])
                nc.sync.dma_start(out=TGT[:, q*FQ:(q+1)*FQ, :],
                                  in_=tgtr[:, q*FQ:(q+1)*FQ, :])

            _kc = {}

            def konst(val):
                if val not in _kc:
                    t = pool.tile([P, 1], F32, tag=f"c{len(_kc)}")
                    nc.gpsimd.memset(t[:, :], val)
                    _kc[val] = t
                return _kc[val][:, :]

            # force table loads early: trig_and_small covers Sin+Abs
            dummy = pool.tile([P, 1], F32, tag="dummy")
            A.activation(out=dummy[:, :], in_=konst(0.0), func=Fn.Sin)
            A.activation(out=dummy[:, :], in_=konst(0.0), func=Fn.Abs)

            pcx, pcy, pw, ph, pth = (PRED[:, :, j] for j in range(5))
            tcx, tcy, tw, th, tth = (TGT[:, :, j] for j in range(5))

            # persistent tiles (halves use disjoint F ranges)
            SZ = pool.tile([P, 4, F], BF16, tag="SZ")      # [a, b, a2, b2]
            TRIGP = pool.tile([P, 2, F], BF16, tag="TRIGP")  # [cp, sp]
            TRIGT = pool.tile([P, 2, F], BF16, tag="TRIGT")  # [ct, st]
            TRIG = pool.tile([P, 2, F], BF16, tag="TRIG")  # [c, s] relative
            DXY0 = pool.tile([P, 2, F], BF16, tag="DXY0")  # [D0x, D0y]
            QUAD = pool.tile([P, 2, 2, F], BF16, tag="QUAD")    # TP4/DP4 scratch
            QUAD2 = pool.tile([P, 2, 2, F], BF16, tag="QUAD2")  # UVP4 scratch
            DD4 = pool.tile([P, 4, F], BF16, tag="DD4")    # [dx, dy, dy, -dx]
            UV2 = pool.tile([P, 2, F], BF16, tag="UV2")    # [-v, -u]
            DTH = pool.tile([P, F], F32, tag="DTH")        # t1 - t2
            ADTH = pool.tile([P, F], BF16, tag="ADTH")     # |t1 - t2|
            Q2 = pool.tile([P, 2, F], BF16, tag="Q2")      # [q2, -q1]
            R4 = pool.tile([P, 4, F], BF16, tag="R4")      # [r1, r2, -r4, r3]
            XP8 = pool.tile([P, 4, 2, F], BF16, tag="XP8")
            INV6 = pool.tile([P, 3, 2, F], BF16, tag="INV6")
            P6 = pool.tile([P, 6, F], BF16, tag="P6")
            AB6 = pool.tile([P, 6, F], BF16, tag="AB6")
            NP = pool.tile([P, 2, 2, F], BF16, tag="NP")
            NT = pool.tile([P, 4, 2, F], BF16, tag="NT")
            MP = pool.tile([P, 2, 2, F], BF16, tag="MP")
            MT = pool.tile([P, 4, 2, F], BF16, tag="MT")
            LOP = pool.tile([P, 2, 2, F], BF16, tag="LOP")
            HIP = pool.tile([P, 2, 2, F], BF16, tag="HIP")
            LOT = pool.tile([P, 4, 2, F], BF16, tag="LOT")
            HIT = pool.tile([P, 4, 2, F], BF16, tag="HIT")
            LPE = pool.tile([P, 2, F], BF16, tag="LPE")
            HPE = pool.tile([P, 2, F], BF16, tag="HPE")
            LTE = pool.tile([P, 2, 2, F], BF16, tag="LTE")
            HTE = pool.tile([P, 2, 2, F], BF16, tag="HTE")
            LPM = pool.tile([P, 2, F], BF16, tag="LPM")
            HPM = pool.tile([P, 2, F], BF16, tag="HPM")
            DTP = pool.tile([P, 2, F], BF16, tag="DTP")
            LTM = pool.tile([P, 2, 2, F], BF16, tag="LTM")
            HTM = pool.tile([P, 2, 2, F], BF16, tag="HTM")
            DTT = pool.tile([P, 2, 2, F], BF16, tag="DTT")
            SMT = pool.tile([P, 2, 2, F], BF16, tag="SMT")
            HV2 = pool.tile([P, 2, F], BF16, tag="HV2")    # [a2s/2, -b2s/2]
            GXY2 = pool.tile([P, 2, F], BF16, tag="GXY2")
            ZT = pool.tile([P, 2, 2, F], BF16, tag="QUAD2")     # reuse (UVP4 dead)
            CONTRIB = pool.tile([P, 2, 2, F], BF16, tag="MP")   # reuse (MP dead)
            CK2 = pool.tile([P, 2, F], BF16, tag="CK2")
            SPDT = pool.tile([P, F], BF16, tag="SPDT")
            AB = pool.tile([P, F], BF16, tag="AB")         # a*b
            S12 = pool.tile([P, F], F32, tag="S12")
            AR1 = pool.tile([P, F], BF16, tag="AR1")
            AR2 = pool.tile([P, F], BF16, tag="AR2")
            OVP = pool.tile([P, F], F32, tag="OVP")
            SCK = pool.tile([P, F], F32, tag="SCK")
            OV = pool.tile([P, F], F32, tag="OV")
            OVC = pool.tile([P, F], F32, tag="OVC")
            Q2D = pool.tile([P, F], F32, tag="Q2D")
            LNO = pool.tile([P, F], F32, tag="LNO")
            LND = pool.tile([P, F], F32, tag="LND")
            DF = pool.tile([P, F], BF16, tag="DF")
            LG = pool.tile([P, F], F32, tag="LG")
            ACC = pool.tile([P, 4], F32, tag="ACC")

            def phaseA_pred(lo, hi):
                # a = w1/2, b = h1/2  (strided f32 reads, Pool)
                V.tensor_scalar(SZ[:, 0:2, lo:hi],
                                PRED[:, lo:hi, 2:4].rearrange("p f c -> p c f"),
                                0.5, 0.0, Op.mult, Op.add)
                # cp = cos(t1) = Sin(pi/2 - t1); sp = -sin(t1) = Sin(-t1)
                A.activation(out=TRIGP[:, 0, lo:hi], in_=pth[:, lo:hi], func=Fn.Sin,
                             bias=konst(HPI), scale=-1.0)
                A.activation(out=TRIGP[:, 1, lo:hi], in_=pth[:, lo:hi], func=Fn.Sin,
                             scale=-1.0)


            def phaseA_tgt(lo, hi):
                # relative trig FIRST (gates XP8, the critical path):
                # s = sin(t1-t2) (conv: -sin(dtheta)), c = Sin(pi/2 - |dtheta|)
                V.tensor_tensor(DTH[:, lo:hi], pth[:, lo:hi], tth[:, lo:hi], Op.subtract)
                A.activation(out=TRIG[:, 1, lo:hi], in_=DTH[:, lo:hi], func=Fn.Sin)
                A.activation(out=ADTH[:, lo:hi], in_=DTH[:, lo:hi], func=Fn.Abs)
                A.activation(out=TRIG[:, 0, lo:hi], in_=ADTH[:, lo:hi], func=Fn.Sin,
                             bias=konst(HPI), scale=-1.0)
                V.tensor_scalar(SZ[:, 2:4, lo:hi],
                                TGT[:, lo:hi, 2:4].rearrange("p f c -> p c f"),
                                0.5, 0.0, Op.mult, Op.add)
                A.activation(out=TRIGT[:, 0, lo:hi], in_=tth[:, lo:hi], func=Fn.Sin,
                             bias=konst(HPI), scale=-1.0)
                A.activation(out=TRIGT[:, 1, lo:hi], in_=tth[:, lo:hi], func=Fn.Sin,
                             scale=-1.0)

                # D0 = tgt center - pred center (f32 -> bf16), on Pool
                G.tensor_tensor(DXY0[:, :, lo:hi],
                                TGT[:, lo:hi, 0:2].rearrange("p f c -> p c f"),
                                PRED[:, lo:hi, 0:2].rearrange("p f c -> p c f"),
                                Op.subtract)
                # areas on Pool
                G.tensor_tensor(AR1[:, lo:hi], pw[:, lo:hi], ph[:, lo:hi], Op.mult)
                G.tensor_tensor(AR2[:, lo:hi], tw[:, lo:hi], th[:, lo:hi], Op.mult)
                G.tensor_tensor(S12[:, lo:hi], AR1[:, lo:hi], AR2[:, lo:hi], Op.add)

            def core(lo, hi):
                w = hi - lo
                QF = QUAD[:, :, :, :].rearrange("p a b f -> p (a b) f")
                # XP8 = [a,b,a2,b2] x [c,s]
                lxp = SZ[:, :, lo:hi].unsqueeze(2).broadcast_to([P, 4, 2, w])
                rxp = TRIG[:, :, lo:hi].unsqueeze(1).broadcast_to([P, 4, 2, w])
                V.tensor_tensor(XP8[:, :, :, lo:hi], lxp, rxp, Op.mult)
                XPf = XP8[:, :, :, :].rearrange("p a b f -> p (a b) f")
                INVf = INV6[:, :, :, :].rearrange("p a b f -> p (a b) f")

                # DP4 = [D0x*cp, D0x*sp, D0y*cp, D0y*sp]
                ldp = DXY0[:, :, lo:hi].unsqueeze(2).broadcast_to([P, 2, 2, w])
                rdp = TRIGP[:, 0:2, lo:hi].unsqueeze(1).broadcast_to([P, 2, 2, w])
                V.tensor_tensor(QUAD[:, :, :, lo:hi], ldp, rdp, Op.mult)
                # DD3 = [dx, dy, -dx]; fam windows [0:2]=(dx,dy), [1:3]=(dy,-dx)
                V.tensor_tensor(DD4[:, 0, lo:hi], QF[:, 0, lo:hi], QF[:, 3, lo:hi], Op.add)
                V.tensor_tensor(DD4[:, 1, lo:hi], QF[:, 2, lo:hi], QF[:, 1, lo:hi], Op.subtract)
                V.tensor_scalar(DD4[:, 2, lo:hi], DD4[:, 0, lo:hi], -1.0, 0.0, Op.mult, Op.add)

                # UVP4 = [D0x*ct, D0x*st, D0y*ct, D0y*st]
                rup = TRIGT[:, 0:2, lo:hi].unsqueeze(1).broadcast_to([P, 2, 2, w])
                V.tensor_tensor(QUAD2[:, :, :, lo:hi], ldp, rup, Op.mult)
                QF2 = QUAD2[:, :, :, :].rearrange("p a b f -> p (a b) f")
                # UV2 = [+v, +u] (pred side is parity-symmetric: flipping UV just
                # relabels E1<->E3 and only dtE1+dtE3 is used, so signs are free)
                V.tensor_tensor(UV2[:, 0, lo:hi], QF2[:, 2, lo:hi], QF2[:, 1, lo:hi], Op.subtract)
                V.tensor_tensor(UV2[:, 1, lo:hi], QF2[:, 0, lo:hi], QF2[:, 3, lo:hi], Op.add)

                # Q2 = [q2, -q1] = [as+bc, bs-ac]  (Pool)
                EQ = V
                EQ.tensor_tensor(Q2[:, 0, lo:hi], XPf[:, 1, lo:hi], XPf[:, 2, lo:hi], Op.add)
                EQ.tensor_tensor(Q2[:, 1, lo:hi], XPf[:, 3, lo:hi], XPf[:, 0, lo:hi], Op.subtract)
                # R4 = [r1, r2, -r4, r3]; adds: [r2, r3] = [a2s, a2c] + [b2c, b2s]
                (G if lo == 0 else V).tensor_tensor(R4[:, 1:4:2, lo:hi], XPf[:, 5:3:-1, lo:hi],
                                XPf[:, 6:8, lo:hi], Op.add)
                # subs: [r1, -r4] = [a2c, b2c] - [b2s, a2s]
                (G if lo == 0 else V).tensor_tensor(R4[:, 0:3:2, lo:hi], XPf[:, 4:7:2, lo:hi],
                                XPf[:, 7:4:-2, lo:hi], Op.subtract)

                # reciprocals of the 6 clip denominators [bc, bs, a2c, a2s, b2c, b2s]
                with nc.allow_low_precision(reason="1/slope in bf16; tolerated"):
                    V.reciprocal(INVf[:, :, lo:hi], XPf[:, 2:8, lo:hi])

                # alphas: P6 = [b2/bc, a2/bs, a/a2c, b/a2s, b/b2c, a/b2s] then |.|
                V.tensor_tensor(P6[:, 0:2, lo:hi], SZ[:, 3:1:-1, lo:hi],
                                INVf[:, 0:2, lo:hi], Op.mult)
                V.tensor_tensor(P6[:, 2:4, lo:hi], SZ[:, 0:2, lo:hi],
                                INVf[:, 2:4, lo:hi], Op.mult)
                V.tensor_tensor(P6[:, 4:6, lo:hi], SZ[:, 1::-1, lo:hi],
                                INVf[:, 4:6, lo:hi], Op.mult)
                A.activation(out=AB6[:, :, lo:hi], in_=P6[:, :, lo:hi], func=Fn.Abs)

                # pred-edge chain on Pool for h0 (Pool has slack), DVE for h1
                E = G if lo == 0 else V
                # numerators: even = Q - U, odd = Q + U
                E.tensor_tensor(NP[:, :, 0, lo:hi], Q2[:, :, lo:hi], UV2[:, :, lo:hi], Op.subtract)
                E.tensor_tensor(NP[:, :, 1, lo:hi], Q2[:, :, lo:hi], UV2[:, :, lo:hi], Op.add)
                ddw = DD4[:, 0:3, lo:hi]  # [dx, dy, -dx]
                EN = V
                # windows: fams (a2c,a2s) use slots 0:2, fams (b2c,b2s) use 1:3
                EN.tensor_tensor(NT[:, 0:2, 0, lo:hi], R4[:, 0:2, lo:hi], ddw[:, 0:2, :], Op.subtract)
                EN.tensor_tensor(NT[:, 2:4, 0, lo:hi], R4[:, 2:4, lo:hi], ddw[:, 1:3, :], Op.subtract)
                EN.tensor_tensor(NT[:, 0:2, 1, lo:hi], R4[:, 0:2, lo:hi], ddw[:, 0:2, :], Op.add)
                EN.tensor_tensor(NT[:, 2:4, 1, lo:hi], R4[:, 2:4, lo:hi], ddw[:, 1:3, :], Op.add)

                # M = N * I
                invp = INV6[:, 0, :, lo:hi].unsqueeze(2).broadcast_to([P, 2, 2, w])
                E.tensor_tensor(MP[:, :, :, lo:hi], NP[:, :, :, lo:hi], invp, Op.mult)
                invt = INVf[:, 2:6, lo:hi].unsqueeze(2).broadcast_to([P, 4, 2, w])
                V.tensor_tensor(MT[:, :, :, lo:hi], NT[:, :, :, lo:hi], invt, Op.mult)

                # LO/HI = M -/+ alpha
                abp = AB6[:, 0:2, lo:hi].unsqueeze(2).broadcast_to([P, 2, 2, w])
                E.tensor_tensor(LOP[:, :, :, lo:hi], MP[:, :, :, lo:hi], abp, Op.subtract)
                E.tensor_tensor(HIP[:, :, :, lo:hi], MP[:, :, :, lo:hi], abp, Op.add)
                abt = AB6[:, 2:6, lo:hi].unsqueeze(2).broadcast_to([P, 4, 2, w])
                ELT = G if lo == 0 else V
                ELT.tensor_tensor(LOT[:, :, :, lo:hi], MT[:, :, :, lo:hi], abt, Op.subtract)
                ELT.tensor_tensor(HIT[:, :, :, lo:hi], MT[:, :, :, lo:hi], abt, Op.add)

            def coreB(lo, hi):
                w = hi - lo
                XPf = XP8[:, :, :, :].rearrange("p a b f -> p (a b) f")
                # target merges first (pred merge waits on Pool for h0)
                V.tensor_tensor(LTE[:, :, :, lo:hi], LOT[:, 0:3:2, :, lo:hi],
                                LOT[:, 1:4:2, :, lo:hi], Op.max)
                V.tensor_tensor(HTE[:, :, :, lo:hi], HIT[:, 0:3:2, :, lo:hi],
                                HIT[:, 1:4:2, :, lo:hi], Op.min)
                # half-scaled clamps: H' = min(h/2,1), L' = max(l/2,0); the missing
                # floor/ceiling cases all yield dt' <= 0, killed by the relu
                V.tensor_scalar(LTM[:, :, :, lo:hi], LTE[:, :, :, lo:hi], 0.5, 0.0, Op.mult, Op.max)
                V.tensor_scalar(HTM[:, :, :, lo:hi], HTE[:, :, :, lo:hi], 0.5, 1.0, Op.mult, Op.min)
                V.tensor_tensor(DTT[:, :, :, lo:hi], HTM[:, :, :, lo:hi], LTM[:, :, :, lo:hi], Op.subtract)
                (G if lo == 0 else V).tensor_tensor(SMT[:, :, :, lo:hi], HTM[:, :, :, lo:hi],
                                                    LTM[:, :, :, lo:hi], Op.add)
                # pred-edge merge (deferred: LOP/HIP may come from Pool); emitted
                # before the DTT relu so DVE has work while ACT runs it
                V.tensor_tensor(LPE[:, :, lo:hi], LOP[:, 0, :, lo:hi], LOP[:, 1, :, lo:hi], Op.max)
                V.tensor_tensor(HPE[:, :, lo:hi], HIP[:, 0, :, lo:hi], HIP[:, 1, :, lo:hi], Op.min)
                if lo == 0:
                    A.activation(out=DTT[:, :, :, lo:hi], in_=DTT[:, :, :, lo:hi], func=Fn.Relu)
                else:
                    V.tensor_scalar(DTT[:, :, :, lo:hi], DTT[:, :, :, lo:hi], 0.0, 0.0, Op.max, Op.add)
                # pred dt via ACT relu chain: dt = relu(2 - relu(2-hi) - relu(lo))
                A.activation(out=HPM[:, :, lo:hi], in_=HPE[:, :, lo:hi], func=Fn.Relu,
                             bias=konst(2.0), scale=-1.0)
                A.activation(out=LPM[:, :, lo:hi], in_=LPE[:, :, lo:hi], func=Fn.Relu)
                (G if lo == 0 else V).tensor_tensor(DTP[:, :, lo:hi], HPM[:, :, lo:hi],
                                                    LPM[:, :, lo:hi], Op.add)
                A.activation(out=DTP[:, :, lo:hi], in_=DTP[:, :, lo:hi], func=Fn.Relu,
                             bias=konst(2.0), scale=-1.0)
                # pred contribution: ab * (dtE1 + dtE3)  (Pool for h0, DVE for h1)
                T2 = G
                A.activation(out=AB[:, lo:hi], in_=AR1[:, lo:hi], func=Fn.Copy, scale=0.25)
                T2.tensor_tensor(SPDT[:, lo:hi], DTP[:, 0, lo:hi], DTP[:, 1, lo:hi], Op.add)
                T2.tensor_tensor(OVP[:, lo:hi], AB[:, lo:hi], SPDT[:, lo:hi], Op.mult)

                # target contribution: X0 = -Mx*gx  =>  per-edge integral
                # dt*(gy*X0 + gx*gy*SM/2) = dt*gxy*(SM/2 - Mx)
                # HV2 = [a2s, -b2s]; GXY2 = [a2c*a2s, -b2s*b2c]
                A.activation(out=HV2[:, 0, lo:hi], in_=XPf[:, 5, lo:hi], func=Fn.Copy, scale=2.0)
                A.activation(out=HV2[:, 1, lo:hi], in_=XPf[:, 7, lo:hi], func=Fn.Copy, scale=-2.0)
                G.tensor_tensor(GXY2[:, :, lo:hi], XPf[:, 4:7:2, lo:hi], HV2[:, :, lo:hi], Op.mult)
                EZ = V
                EZ.tensor_tensor(ZT[:, :, :, lo:hi], SMT[:, :, :, lo:hi],
                                MT[:, 0:4:3, :, lo:hi], Op.subtract)
                V.tensor_tensor(ZT[:, :, :, lo:hi], ZT[:, :, :, lo:hi], DTT[:, :, :, lo:hi], Op.mult)
                gxy = GXY2[:, :, lo:hi].unsqueeze(2).broadcast_to([P, 2, 2, w])
                V.tensor_tensor(CONTRIB[:, :, :, lo:hi], ZT[:, :, :, lo:hi], gxy, Op.mult)
                T = G if lo == 0 else V
                T.tensor_tensor(CK2[:, :, lo:hi], CONTRIB[:, 0, :, lo:hi],
                                CONTRIB[:, 1, :, lo:hi], Op.add)
                T.tensor_tensor(SCK[:, lo:hi], CK2[:, 0, lo:hi], CK2[:, 1, lo:hi], Op.add)

            def tail(lo, hi, i):
                T = G if i < 3 else V
                V.tensor_tensor(OV[:, lo:hi], OVP[:, lo:hi], SCK[:, lo:hi], Op.add)
                V.tensor_scalar(OVC[:, lo:hi], OV[:, lo:hi], 8e-35, 1e38, Op.max, Op.min)
                V.tensor_tensor(Q2D[:, lo:hi], S12[:, lo:hi], OV[:, lo:hi], Op.subtract)
                A.activation(out=LNO[:, lo:hi], in_=OVC[:, lo:hi], func=Fn.Ln)
                A.activation(out=LND[:, lo:hi], in_=Q2D[:, lo:hi], func=Fn.Ln, bias=konst(EPS))
                T.tensor_tensor(DF[:, lo:hi], LNO[:, lo:hi], LND[:, lo:hi], Op.subtract)
                V.tensor_scalar(LG[:, lo:hi], DF[:, lo:hi], LNEPS, 0.0,
                                Op.max, Op.add, accum_out=ACC[:, i:i+1])

            for q in range(4):
                phaseA_pred(q * FQ, (q + 1) * FQ)
                phaseA_tgt(q * FQ, (q + 1) * FQ)
            core(0, FH)
            core(FH, F)
            # prefetch natural_log table after last Abs emission
            A.activation(out=dummy[:, :], in_=konst(1.0), func=Fn.Ln)
            coreB(0, FH)
            coreB(FH, F)
            tail(0, FQ, 0)
            tail(FQ, FH, 1)
            nc.sync.dma_start(out=out_h[:, 0:2], in_=ACC[:, 0:2])
            tail(FH, FH + FQ, 2)
            tail(FH + FQ, F, 3)
            nc.sync.dma_start(out=out_h[:, 2:4], in_=ACC[:, 2:4])

    nc.compile()
    return nc


_NC = None


def _get_nc():
    global _NC
    if _NC is None:
        _NC = _build()
    return _NC


class _Runner:
    def __init__(self, nc):
        import jax
        from jax.sharding import Mesh, PartitionSpec
        try:
            from jax.experimental.shard_map import shard_map
        except ImportError:
            from jax.shard_map import shard_map
        from concourse import bass2jax, mybir as mb

        bass2jax.install_neuronx_cc_hook()
        self.jax = jax
        partition_name = (nc.partition_id_tensor.name
                          if nc.partition_id_tensor else None)
        in_names, out_names, out_avals, zero_outs = [], [], [], []
        for alloc in nc.m.functions[0].allocations:
            if not isinstance(alloc, mb.MemoryLocationSet):
                continue
            name = alloc.memorylocations[0].name
            if alloc.kind == "ExternalInput":
                if name != partition_name:
                    in_names.append(name)
            elif alloc.kind == "ExternalOutput":
                shape = tuple(alloc.tensor_shape)
                dtype = mb.dt.np(alloc.dtype)
                out_names.append(name)
                out_avals.append(jax.core.ShapedArray(shape, dtype))
                zero_outs.append(np.zeros((NCORES * shape[0],) + shape[1:], dtype))
        self.in_names = list(in_names)
        self.out_names = list(out_names)
        self.zero_outs = zero_outs
        n_params = len(in_names)
        all_names = in_names + out_names
        if partition_name is not None:
            all_names = all_names + [partition_name]

        def _body(*args):
            operands = list(args)
            if partition_name is not None:
                operands.append(bass2jax.partition_id_tensor())
            outs = bass2jax._bass_exec_p.bind(
                *operands,
                out_avals=tuple(out_avals),
                in_names=tuple(all_names),
                out_names=tuple(out_names),
                lowering_input_output_aliases=(),
                sim_require_finite=True,
                sim_require_nnan=True,
                nc=nc,
            )
            return tuple(outs)

        devices = jax.devices()[:NCORES]
        mesh = Mesh(np.asarray(devices), ("core",))
        n_outs = len(out_names)
        self.fn = jax.jit(
            shard_map(_body, mesh=mesh,
                      in_specs=(PartitionSpec("core"),) * (n_params + n_outs),
                      out_specs=(PartitionSpec("core"),) * n_outs,
                      check_rep=False),
            donate_argnums=tuple(range(n_params, n_params + n_outs)),
            keep_unused=True,
        )

    def __call__(self, pred, target):
        ins = {"pred": pred, "target": target}
        args = [ins[n] for n in self.in_names] + [z.copy() for z in self.zero_outs]
        outs = self.fn(*args)
        return [np.asarray(o) for o in outs]


_RUNNER = None


def _get_runner():
    global _RUNNER
    if _RUNNER is None:
        _RUNNER = _Runner(_get_nc())
    return _RUNNER


def kernel(pred: np.ndarray, target: np.ndarray) -> np.ndarray:
    pred = np.ascontiguousarray(np.asarray(pred, dtype=np.float32))
    target = np.ascontiguousarray(np.asarray(target, dtype=np.float32))
    assert pred.shape == (N_TOTAL, 5) and target.shape == (N_TOTAL, 5)

    runner = _get_runner()
    outs = runner(pred, target)
    total = outs[0].astype(np.float64).sum()
    return np.float32(-(total / N_TOTAL))
